# revision 1
# baseline (speedup 1.0000x reference)
"""BitNet FFN (bitlinear158 -> gelu -> bitlinear158) Trainium2 kernel.

Sharding: data-parallel over tokens across 8 cores (1024 tokens/core).
Layout: tokens on the free axis everywhere; weights stationary in the PE.

Math notes (exactness):
  - activation quant ints = round(x * 127 / max|x|)  (the rms-norm cancels)
  - weight quant ternary = clip(round(w / clip(mean|w|,1e-5)), -1, 1)
  - both exactly representable in bf16; PSUM accumulates integer products
    (<= 2^21) exactly in fp32, so the matmuls are exact.
  - per-token output scale alpha = clip(max|x|*sqrt(d)/||x||, 1e-5)
      * clip(mean|w|,1e-5) / 127 applied on PSUM before gelu.
  - round-to-nearest-even via fp32 (t + 1.5*2^23) - 1.5*2^23, matching
    jnp.round; clip(round(t),-1,1) == round(clamp(t, +-1.4999999)).
  - mean|w| needs the full tensor: each core reduces its row-shard, then a
    tiny AllReduce combines the partial sums.
"""

import sys

for _p in ("/opt/trn_rl_repo", "/opt/trn_rl_repo/concourse"):
    if _p not in sys.path:
        sys.path.insert(0, _p)

import numpy as np

import concourse.bass as bass
import concourse.bacc as bacc
import concourse.mybir as mybir
import concourse.tile as tile
from concourse import library_config
from concourse.bass import ts
from concourse.masks import make_identity

F32 = mybir.dt.float32
BF16 = mybir.dt.bfloat16
AX = mybir.AxisListType.X
OP = mybir.AluOpType
AF = mybir.ActivationFunctionType

C_ROUND = 12582912.0  # 1.5 * 2**23 : fp32 RNE rounding constant
W_CLIP = 1.4999999    # round(clamp(t, +-W_CLIP)) == clip(round(t), -1, 1)
N_CORES = 8


def build_bitnet(D, I, T, n_cores=N_CORES, gelu_mode="gelu"):
    """Per-core SPMD Bass program.

    Per-core I/O: xT [D,T] f32 (token shard, transposed), w1T [D,I] f32 and
    w2T [I,D] f32 (full transposed weights), w1s [D/n,I] / w2s [I/n,D]
    (this core's rows, for the mean|w| partial) -> outT [D,T] f32.
    """
    KD = D // 128   # d tiles (layer-1 contraction; layer-2 output rows)
    KI = I // 128   # inner tiles
    TH = T // 2     # matmul moving free dim
    TJ = T // 128   # token tiles for stats transposes
    JD2 = D // 256  # paired output-column strips in layer 2
    K2H = KI // 2   # half of inner tiles (layer-2 weight streaming)
    R1 = D // n_cores   # w1 shard rows per core
    R2 = I // n_cores   # w2 shard rows per core
    A1 = (R1 + 127) // 128
    A2 = (R2 + 127) // 128
    inv_cnt = 1.0 / float(D * I)
    sqrt_d = float(np.sqrt(np.float64(D)))
    sqrt_i = float(np.sqrt(np.float64(I)))

    nc = bacc.Bacc("TRN2", num_devices=n_cores)

    xT = nc.dram_tensor("xT", [D, T], F32, kind="ExternalInput")
    w1T = nc.dram_tensor("w1T", [D, I], F32, kind="ExternalInput")
    w2T = nc.dram_tensor("w2T", [I, D], F32, kind="ExternalInput")
    w1s = nc.dram_tensor("w1s", [D // n_cores, I], F32, kind="ExternalInput")
    w2s = nc.dram_tensor("w2s", [I // n_cores, D], F32, kind="ExternalInput")
    outT = nc.dram_tensor("outT", [D, T], F32, kind="ExternalOutput")

    h_dram = nc.dram_tensor("h_scratch", [I, T], F32, kind="Internal")
    w1ag_in = nc.dram_tensor("w1ag_in", [R1, I], BF16, kind="Internal")
    w1q_dram = nc.dram_tensor("w1q_ag", [D, I], BF16, kind="Internal",
                              addr_space="Shared")
    w2ag_in = nc.dram_tensor("w2ag_in", [R2, D], BF16, kind="Internal")
    w2q_dram = nc.dram_tensor("w2q_ag", [I, D], BF16, kind="Internal",
                              addr_space="Shared")
    ar_in = nc.dram_tensor("ar_in", [8], F32, kind="Internal")
    ar_out = nc.dram_tensor("ar_out", [8], F32, kind="Internal",
                            addr_space="Shared")
    stat_dram = nc.dram_tensor("stat_dram", [6, T], F32, kind="Internal")
    srow_v = stat_dram.ap()                                     # [6, T]
    stok_v = stat_dram.ap().rearrange("r (j p) -> r p j", p=128)  # [6,128,TJ]

    xT_t = xT.ap().rearrange("(k p) t -> k p t", p=128)           # [KD,128,T]
    w1_t = w1T.ap().rearrange("(k p) (i j) -> i p k j", p=128, j=128)
    w1s_ap = w1s.ap()
    w2s_ap = w2s.ap()
    w1q_t = w1q_dram.ap().rearrange("(k p) (i j) -> i p k j", p=128, j=128)
    w2q_r = w2q_dram.ap().rearrange("(k p) (m c) -> m p k c", p=128, c=256)
    h_w = h_dram.ap().rearrange("(k p) t -> k p t", p=128)
    out_w = outT.ap().rearrange("(k p) t -> k p t", p=128)

    with tile.TileContext(nc) as tc:
        with (
            tc.tile_pool(name="glob", bufs=1) as glob,
            tc.tile_pool(name="psum", bufs=8, space="PSUM") as psum,
            tc.tile_pool(name="stats", bufs=1) as stats,
        ):
            # --- persistent constants & small tiles ---
            ident = glob.tile([128, 128], F32)
            make_identity(nc, ident)
            wsc = glob.tile([128, 4], F32)   # cols: s1, s2, mclip1, mclip2
            qs1_b = glob.tile([128, T], F32, tag="qsb")
            al1_b = glob.tile([128, T], F32, tag="alb")

            # stats layout shuffles go through DRAM rows: token t = 128*j + p

            def part_reduce(acc, res, op):
                # reduce [128, T] over partitions -> res [128, TJ] tok-part
                for j in range(TJ):
                    trp = psum.tile([128, 128], F32, tag="b", name="trp")
                    nc.tensor.transpose(trp[:, :], acc[:, ts(j, 128)],
                                        ident[:, :])
                    nc.vector.tensor_reduce(
                        out=res[:, j:j + 1], in_=trp[:, :], axis=AX, op=op)

            def finalize_stats(Mx, ssq, mclip_col, sqrt_dim, qs_b, al_b, r0):
                """Mx/ssq [128,TJ] tok-part absmax / sumsq.
                Builds qs_b = 127/max|x| and al_b = per-token dequant scale,
                both broadcast to [128, T]. r0: base row in stat_dram."""
                nrm = stats.tile([128, TJ], F32, name="nrm")
                nc.vector.tensor_scalar(nrm, ssq, 1e-38, None, OP.max)
                nc.scalar.activation(nrm, nrm, AF.Sqrt)
                nc.vector.tensor_scalar(nrm, nrm, 1e-12, None, OP.max)
                inv_n = stats.tile([128, TJ], F32, name="inv_n")
                nc.vector.reciprocal(inv_n, nrm)
                al = stats.tile([128, TJ], F32, name="al")
                nc.vector.tensor_tensor(al, Mx, inv_n, OP.mult)
                nc.vector.tensor_scalar(al, al, sqrt_dim, 1e-5, OP.mult, OP.max)
                nc.vector.tensor_scalar(al, al, wsc[:, mclip_col:mclip_col + 1],
                                        1.0 / 127.0, OP.mult, OP.mult)
                qs = stats.tile([128, TJ], F32, name="qs")
                nc.vector.tensor_scalar(qs, Mx, 1e-30, None, OP.max)
                nc.vector.reciprocal(qs, qs)
                nc.vector.tensor_scalar(qs, qs, 127.0, None, OP.mult)
                nc.sync.dma_start(out=stok_v[r0 + 1], in_=qs[:, :])
                nc.sync.dma_start(out=stok_v[r0 + 2], in_=al[:, :])
                qrow = stats.tile([1, T], F32, name="qrow")
                arow = stats.tile([1, T], F32, name="arow")
                nc.sync.dma_start(out=qrow[:, :], in_=srow_v[r0 + 1:r0 + 2, :])
                nc.sync.dma_start(out=arow[:, :], in_=srow_v[r0 + 2:r0 + 3, :])
                nc.gpsimd.partition_broadcast(qs_b[:, :], qrow[:, :])
                nc.gpsimd.partition_broadcast(al_b[:, :], arow[:, :])

            # ========= Stage A: weight scale partials + AllReduce =========
            with tc.tile_pool(name="wredp", bufs=2) as wredp:
                wps = stats.tile([128, A1 + A2], F32)
                if R1 % 128 or R2 % 128:  # partial chunks need zero padding
                    nc.vector.memset(wps, 0.0)
                for a in range(A1):
                    pp = min(128, R1 - 128 * a)
                    wtmp = wredp.tile([128, I], F32, tag="wred", name="wtmp")
                    nc.sync.dma_start(out=wtmp[:pp, :],
                                      in_=w1s_ap[128 * a:128 * a + pp, :])
                    nc.vector.tensor_reduce(
                        out=wps[:pp, a:a + 1], in_=wtmp[:pp, :], axis=AX,
                        op=OP.add, apply_absolute_value=True)
                for a in range(A2):
                    pp = min(128, R2 - 128 * a)
                    wtmp2 = wredp.tile([128, I], F32, tag="wred", name="wtmp2")
                    nc.sync.dma_start(out=wtmp2[:pp, :D],
                                      in_=w2s_ap[128 * a:128 * a + pp, :])
                    nc.vector.tensor_reduce(
                        out=wps[:pp, A1 + a:A1 + a + 1], in_=wtmp2[:pp, :D],
                        axis=AX, op=OP.add, apply_absolute_value=True)
                wpad = stats.tile([128, 128], F32)
                nc.vector.memset(wpad, 0.0)
                nc.vector.reduce_sum(wpad[:, 0:1], wps[:, 0:A1], axis=AX)
                nc.vector.reduce_sum(wpad[:, 1:2], wps[:, A1:A1 + A2], axis=AX)
                trw = psum.tile([128, 128], F32, tag="b", name="trw")
                nc.tensor.transpose(trw[:, :], wpad[:, :], ident[:, :])
                wred = stats.tile([8, 1], F32)
                nc.vector.memset(wred, 0.0)
                nc.vector.reduce_sum(wred[0:2, :], trw[0:2, :], axis=AX)
                nc.sync.dma_start(out=ar_in.ap()[0:8], in_=wred[:, :])
                nc.gpsimd.collective_compute(
                    "AllReduce", OP.add,
                    replica_groups=[list(range(n_cores))],
                    ins=[ar_in.ap().opt()], outs=[ar_out.ap().opt()])
                wrow = stats.tile([1, 2], F32)
                nc.sync.dma_start(out=wrow[:, :], in_=ar_out.ap()[0:2])
                mrow = stats.tile([1, 4], F32)
                nc.vector.tensor_scalar(mrow[:, 2:4], wrow[:, :], inv_cnt,
                                        1e-5, OP.mult, OP.max)
                nc.vector.reciprocal(mrow[:, 0:2], mrow[:, 2:4])
                nc.gpsimd.partition_broadcast(wsc[:, :], mrow[:, :])

                # quantize this core's shards (ternary bf16), then AllGather
                def quant_shard(src_ap, rows, width, scol, dst_ap):
                    for a in range((rows + 127) // 128):
                        pp = min(128, rows - 128 * a)
                        wqf = wredp.tile([128, I], F32, tag="wred",
                                         name="wqf")
                        nc.sync.dma_start(
                            out=wqf[:pp, :width],
                            in_=src_ap[128 * a:128 * a + pp, :])
                        nc.scalar.activation(wqf[:pp, :width],
                                             wqf[:pp, :width], AF.Copy,
                                             scale=wsc[:pp, scol:scol + 1])
                        nc.vector.tensor_scalar(wqf[:pp, :width],
                                                wqf[:pp, :width], W_CLIP,
                                                -W_CLIP, OP.min, OP.max)
                        wqb = wredp.tile([128, I], BF16, tag="wqb",
                                         name="wqb")
                        nc.vector.tensor_scalar(wqb[:pp, :width],
                                                wqf[:pp, :width], C_ROUND,
                                                C_ROUND, OP.add, OP.subtract)
                        nc.sync.dma_start(
                            out=dst_ap[128 * a:128 * a + pp, :],
                            in_=wqb[:pp, :width])

                quant_shard(w1s_ap, R1, I, 0, w1ag_in.ap())
                nc.gpsimd.collective_compute(
                    "AllGather", OP.bypass,
                    replica_groups=[list(range(n_cores))],
                    ins=[w1ag_in.ap().opt()], outs=[w1q_dram.ap().opt()])
                quant_shard(w2s_ap, R2, D, 1, w2ag_in.ap())
                nc.gpsimd.collective_compute(
                    "AllGather", OP.bypass,
                    replica_groups=[list(range(n_cores))],
                    ins=[w2ag_in.ap().opt()], outs=[w2q_dram.ap().opt()])

            with tc.tile_pool(name="bc", bufs=2) as bc:
                # ================= Stage B: x stats + quant =================
                am1p = stats.tile([128, T], F32, tag="amp", name="am1p")
                am1n = stats.tile([128, T], F32, tag="amn", name="am1n")
                sq1 = stats.tile([128, T], F32, tag="sq", name="sq1")
                for k in range(KD):
                    xk = bc.tile([128, T], F32, tag="xk", name="xk")
                    nc.sync.dma_start(out=xk[:, :], in_=xT_t[k])
                    if k == 0:
                        nc.vector.tensor_copy(am1p, xk)
                        nc.vector.tensor_copy(am1n, xk)
                    else:
                        nc.vector.tensor_tensor(am1p, xk, am1p, OP.max)
                        nc.vector.tensor_tensor(am1n, xk, am1n, OP.min)
                    xsq = bc.tile([128, T], BF16, tag="xsq", name="xsq")
                    nc.scalar.activation(xsq, xk, AF.Square)
                    if k == 0:
                        nc.vector.tensor_copy(sq1, xsq)
                    else:
                        nc.vector.tensor_tensor(sq1, xsq, sq1, OP.add)
                nc.vector.scalar_tensor_tensor(
                    am1n, am1n, -1.0, am1p, OP.mult, OP.max)
                Mx1 = stats.tile([128, TJ], F32)
                part_reduce(am1n, Mx1, OP.max)
                Sq1 = stats.tile([128, TJ], F32)
                part_reduce(sq1, Sq1, OP.add)
                finalize_stats(Mx1, Sq1, 2, sqrt_d, qs1_b, al1_b, 0)

                xqT = bc.tile([128, KD, T], BF16, tag="xqT", bufs=1,
                              name="xqT")
                for k in range(KD):
                    xk2 = bc.tile([128, T], F32, tag="xk", name="xk2")
                    nc.sync.dma_start(out=xk2[:, :], in_=xT_t[k])
                    nc.vector.tensor_tensor(xk2, xk2, qs1_b, OP.mult)
                    nc.vector.tensor_scalar(xqT[:, k, :], xk2, C_ROUND,
                                            C_ROUND, OP.add, OP.subtract)

                # ===== Stage C: layer 1 + h stats + w2 quant (interleaved) ====
                am2p = stats.tile([128, T], F32, tag="amp", name="am2p")
                am2n = stats.tile([128, T], F32, tag="amn", name="am2n")
                sq2 = stats.tile([128, T], F32, tag="sq2", name="sq2")
                n_jit = min(16, KI)  # strips quantized locally while the
                for i in range(KI):  # w1q AllGather is still in flight
                    w1q = bc.tile([128, KD, 128], BF16, tag="w1q", name="w1q")
                    if i < n_jit:
                        w1f = bc.tile([128, KD, 128], F32, tag="w1f",
                                      name="w1f")
                        nc.sync.dma_start(out=w1f[:, :, :], in_=w1_t[i])
                        w1ff = w1f.rearrange("p k j -> p (k j)")
                        nc.scalar.activation(w1ff, w1ff, AF.Copy,
                                             scale=wsc[:, 0:1])
                        nc.vector.tensor_scalar(w1ff, w1ff, W_CLIP, -W_CLIP,
                                                OP.min, OP.max)
                        nc.vector.tensor_scalar(
                            w1q.rearrange("p k j -> p (k j)"), w1ff, C_ROUND,
                            C_ROUND, OP.add, OP.subtract)
                    else:
                        nc.sync.dma_start(out=w1q[:, :, :], in_=w1q_t[i])
                    hpsA = psum.tile([128, TH], F32, tag="b", name="hpsA")
                    hpsB = psum.tile([128, TH], F32, tag="b", name="hpsB")
                    for k in range(KD):
                        nc.tensor.matmul(hpsA[:, :], w1q[:, k, :],
                                         xqT[:, k, 0:TH],
                                         start=(k == 0), stop=(k == KD - 1))
                    for k in range(KD):
                        nc.tensor.matmul(hpsB[:, :], w1q[:, k, :],
                                         xqT[:, k, TH:T],
                                         start=(k == 0), stop=(k == KD - 1))
                    nc.vector.tensor_tensor(hpsA, hpsA, al1_b[:, 0:TH], OP.mult)
                    nc.vector.tensor_tensor(hpsB, hpsB, al1_b[:, TH:T], OP.mult)
                    h_sb = bc.tile([128, T], F32, tag="h", bufs=3, name="h_sb")
                    if gelu_mode == "gelu":
                        nc.scalar.activation(h_sb[:, 0:TH], hpsA, AF.Gelu)
                        nc.scalar.activation(h_sb[:, TH:T], hpsB, AF.Gelu)
                    else:  # sigmoid-gelu (CoreSim lacks Gelu/Erf tables)
                        gs = bc.tile([128, T], F32, tag="gsig", name="gs")
                        nc.scalar.activation(gs[:, 0:TH], hpsA, AF.Sigmoid,
                                             scale=1.702)
                        nc.scalar.activation(gs[:, TH:T], hpsB, AF.Sigmoid,
                                             scale=1.702)
                        nc.vector.tensor_tensor(h_sb[:, 0:TH], gs[:, 0:TH],
                                                hpsA, OP.mult)
                        nc.vector.tensor_tensor(h_sb[:, TH:T], gs[:, TH:T],
                                                hpsB, OP.mult)
                    nc.sync.dma_start(out=h_w[i], in_=h_sb[:, :])
                    if i == 0:
                        nc.vector.tensor_copy(am2p, h_sb)
                        nc.vector.tensor_copy(am2n, h_sb)
                    else:
                        nc.vector.tensor_tensor(am2p, h_sb, am2p, OP.max)
                        nc.vector.tensor_tensor(am2n, h_sb, am2n, OP.min)
                    hsq = bc.tile([128, T], BF16, tag="hsq", name="hsq")
                    nc.scalar.activation(hsq, h_sb, AF.Square)
                    if i == 0:
                        nc.vector.tensor_copy(sq2, hsq)
                    else:
                        nc.vector.tensor_tensor(sq2, hsq, sq2, OP.add)

                # ---- mid stats finalize ----
                qs2_b = glob.tile([128, T], F32, tag="qsb", name="qs2_b")
                al2_b = glob.tile([128, T], F32, tag="alb", name="al2_b")
                nc.vector.scalar_tensor_tensor(
                    am2n, am2n, -1.0, am2p, OP.mult, OP.max)
                Mx2 = stats.tile([128, TJ], F32, name="Mx2")
                part_reduce(am2n, Mx2, OP.max)
                Sq2 = stats.tile([128, TJ], F32, name="Sq2")
                part_reduce(sq2, Sq2, OP.add)
                finalize_stats(Mx2, Sq2, 3, sqrt_i, qs2_b, al2_b, 3)

            # ================= Stage D: quantize h, layer 2 =================
            with tc.tile_pool(name="l2", bufs=2) as l2:
                hqT = l2.tile([128, KI, T], BF16, tag="hqT", bufs=1,
                              name="hqT")
                for k2 in range(KI):
                    hk = l2.tile([128, T], F32, tag="hrd", name="hk")
                    nc.sync.dma_start(out=hk[:, :], in_=h_w[k2])
                    nc.vector.tensor_tensor(hk, hk, qs2_b, OP.mult)
                    nc.vector.tensor_scalar(hqT[:, k2, :], hk, C_ROUND,
                                            C_ROUND, OP.add, OP.subtract)
                for m in range(JD2):
                    pbank = [psum.tile([128, TH], F32, tag="b",
                                       name=f"psb{q}") for q in range(4)]
                    for kh in range(2):
                        w2qs = l2.tile([128, K2H, 256], BF16, tag="w2s",
                                       bufs=2, name="w2qs")
                        nc.sync.dma_start(
                            out=w2qs[:, :, :],
                            in_=w2q_r[m][:, ts(kh, K2H), :])
                        for kk in range(K2H):
                            k2 = kh * K2H + kk
                            first = (k2 == 0)
                            last = (k2 == KI - 1)
                            nc.tensor.matmul(pbank[0][:, :], w2qs[:, kk, 0:128],
                                             hqT[:, k2, 0:TH],
                                             start=first, stop=last)
                            nc.tensor.matmul(pbank[1][:, :], w2qs[:, kk, 0:128],
                                             hqT[:, k2, TH:T],
                                             start=first, stop=last)
                            nc.tensor.matmul(pbank[2][:, :],
                                             w2qs[:, kk, 128:256],
                                             hqT[:, k2, 0:TH],
                                             start=first, stop=last)
                            nc.tensor.matmul(pbank[3][:, :],
                                             w2qs[:, kk, 128:256],
                                             hqT[:, k2, TH:T],
                                             start=first, stop=last)
                    for jcol in range(2):
                        for half in range(2):
                            ob = l2.tile([128, TH], F32, tag="ob", bufs=2,
                                         name="ob")
                            nc.vector.tensor_tensor(
                                ob, pbank[2 * jcol + half],
                                al2_b[:, ts(half, TH)], OP.mult)
                            nc.sync.dma_start(
                                out=out_w[2 * m + jcol][:, ts(half, TH)],
                                in_=ob[:, :])

    nc.compile()  # Bacc passes: EVSEM multi-wait lowering, library loads,
    return nc     # extended-ISA codegen, nop fusion, register alloc


_NC_CACHE = {}


def _get_nc(D, I, T, n_cores):
    key = (D, I, T, n_cores)
    if key not in _NC_CACHE:
        _NC_CACHE[key] = build_bitnet(D, I, T, n_cores)
    return _NC_CACHE[key]


def make_in_maps(x, w1, w2, n_cores=N_CORES):
    """Host-side sharding/layout only (transpose + slicing, no arithmetic)."""
    xf = np.ascontiguousarray(np.asarray(x, dtype=np.float32)).reshape(
        -1, x.shape[-1])
    D = xf.shape[1]
    I = w1.shape[0]
    T = xf.shape[0] // n_cores
    w1T = np.ascontiguousarray(np.asarray(w1, dtype=np.float32).T)  # [D, I]
    w2T = np.ascontiguousarray(np.asarray(w2, dtype=np.float32).T)  # [I, D]
    in_maps = []
    for c in range(n_cores):
        xTc = np.ascontiguousarray(xf[c * T:(c + 1) * T].T)  # [D, T]
        in_maps.append({
            "xT": xTc,
            "w1T": w1T,
            "w2T": w2T,
            "w1s": np.ascontiguousarray(
                w1T[c * (D // n_cores):(c + 1) * (D // n_cores)]),
            "w2s": np.ascontiguousarray(
                w2T[c * (I // n_cores):(c + 1) * (I // n_cores)]),
        })
    return in_maps, (D, I, T)


def run_spmd(x, w1, w2, trace=False, **kwargs):
    from concourse.bass_utils import run_bass_kernel_spmd

    B, S, D = x.shape
    in_maps, (D, I, T) = make_in_maps(x, w1, w2, N_CORES)
    nc = _get_nc(D, I, T, N_CORES)
    res = run_bass_kernel_spmd(nc, in_maps, core_ids=list(range(N_CORES)),
                               trace=trace, **kwargs)
    outs = [res.results[c]["outT"].T for c in range(N_CORES)]  # each [T, D]
    out = np.concatenate(outs, axis=0).reshape(B, S, D)
    return np.ascontiguousarray(out, dtype=np.float32), res


def kernel(x, w1, w2):
    out, _ = run_spmd(x, w1, w2, trace=False)
    return out



# revision 2
# speedup vs baseline: 1.0343x; 1.0343x over previous
"""BitNet FFN Trainium2 kernel, v4.

Data-parallel over tokens (1024/core). Ternary weights in fp8e4 (exact),
int8 activations in bf16 (exact); matmuls mixed fp8 lhsT x bf16 rhs
(HW-validated exact). Per-token dequant scale applied to PSUM on DVE;
mean|w| rides the ACT Gelu scale input.

Engine assignment (strict-FIFO queues make this the core design axis):
  DVE:    x stats trees, xq build, JIT clamp/round, PSUM alpha/max/sq,
          hq mult, L2 out scale
  ACT:    x abs/sq, JIT scale-copy, gelu, squares
  GpSimd: |w| shard reduces, AllReduce pack, weight-scale bcasts, shard
          ternary quant (divide recipe), collectives, hq round
  PE:     matmuls (same-bank chains), stat transposes
"""

import sys

for _p in ("/opt/trn_rl_repo", "/opt/trn_rl_repo/concourse"):
    if _p not in sys.path:
        sys.path.insert(0, _p)

import numpy as np

import concourse.bass as bass
import concourse.bacc as bacc
import concourse.mybir as mybir
import concourse.tile as tile
from concourse import bass_isa
from concourse.bass import ts
from concourse.masks import make_identity

F32 = mybir.dt.float32
BF16 = mybir.dt.bfloat16
FP8 = mybir.dt.float8e4
AX = mybir.AxisListType.X
OP = mybir.AluOpType
AF = mybir.ActivationFunctionType

C_ROUND = 12582912.0  # 1.5 * 2**23 : fp32 RNE rounding constant
W_CLIP = 1.4999999    # round(clamp(t, +-W_CLIP)) == clip(round(t), -1, 1)
N_CORES = 8
J_JIT = 10            # L1 strips quantized locally while AllGather in flight


def build_bitnet(D, I, T, n_cores=N_CORES, gelu_mode="gelu", j_jit=J_JIT):
    KD = D // 128        # contraction tiles for L1
    KI = I // 128        # strips == L2 contraction tiles
    MD = D // 128        # output m-tiles for L2
    TH = T // 2          # token chunk
    TJ = T // 128        # stat columns (full T)
    TJH = TH // 128      # stat columns per chunk
    SO1 = I // n_cores // 128   # own w1 strips
    MO2 = D // 128 // n_cores   # own w2 m-tiles
    MG = 4                      # L2 m-tiles per pass (MG*2 psum banks)
    W2C = max(1, KI // 16)      # w2 shard chunks per m ([128, <=2048] each)
    K2C = KI // W2C
    inv_cnt = 1.0 / float(D * I)
    sqrt_d = float(np.sqrt(np.float64(D)))
    sqrt_i = float(np.sqrt(np.float64(I)))
    j_jit = min(j_jit, KI)

    nc = bacc.Bacc("TRN2", num_devices=n_cores)

    xT = nc.dram_tensor("xT", [D, T], F32, kind="ExternalInput")
    w1T = nc.dram_tensor("w1T", [D, I], F32, kind="ExternalInput")
    w1s = nc.dram_tensor("w1s", [D, I // n_cores], F32, kind="ExternalInput")
    w2s = nc.dram_tensor("w2s", [I, D // n_cores], F32, kind="ExternalInput")
    outT = nc.dram_tensor("outT", [D, T], F32, kind="ExternalOutput")

    h_dram = nc.dram_tensor("h_scratch", [KI, 128, T], F32, kind="Internal")
    w1ag_in = nc.dram_tensor("w1ag_in", [SO1, 128, KD * 128], FP8,
                             kind="Internal")
    w1q = nc.dram_tensor("w1q_ag", [KI, 128, KD * 128], FP8, kind="Internal",
                         addr_space="Shared")
    w2ag_in = nc.dram_tensor("w2ag_in", [MO2, 128, KI * 128], FP8,
                             kind="Internal")
    w2q = nc.dram_tensor("w2q_ag", [MD, 128, KI * 128], FP8, kind="Internal",
                         addr_space="Shared")
    ar_in = nc.dram_tensor("ar_in", [8], F32, kind="Internal")
    ar_out = nc.dram_tensor("ar_out", [8], F32, kind="Internal",
                            addr_space="Shared")
    stat_dram = nc.dram_tensor("stat_dram", [4, T], F32, kind="Internal")
    srow_v = stat_dram.ap()                                       # [4, T]
    stok_v = stat_dram.ap().rearrange("r (j p) -> r p j", p=128)  # [4,128,TJ]

    xT_t = xT.ap().rearrange("(k p) t -> k p t", p=128)           # [KD,128,T]
    w1_t = w1T.ap().rearrange("(k p) (s j) -> s p k j", p=128, j=128)
    w1s_t = w1s.ap().rearrange("(k p) (s j) -> s p k j", p=128, j=128)
    w2s_t = w2s.ap().rearrange("(k p) (m j) -> m p k j", p=128, j=128)
    out_w = outT.ap().rearrange("(m p) t -> m p t", p=128)

    def gelu(out, in_, scale):
        if gelu_mode == "gelu":
            nc.scalar.activation(out, in_, AF.Gelu, scale=scale)
        else:  # CoreSim lacks the Gelu table; sim-only stand-in
            nc.scalar.activation(out, in_, AF.Sigmoid, scale=scale)

    with tile.TileContext(nc) as tc:
        with (
            tc.tile_pool(name="glob", bufs=1) as glob,
            tc.tile_pool(name="stats", bufs=1) as stats,
        ):
            ident = glob.tile([128, 128], F32)
            make_identity(nc, ident)
            ident_b = glob.tile([128, 128], BF16)
            make_identity(nc, ident_b)
            wsc = glob.tile([128, 4], F32)   # cols: mclip1, mclip2, s1, s2
            qs2_b = glob.tile([128, T], F32, name="qs2_b")
            al2_b = glob.tile([128, T], F32, name="al2_b")

            def part_reduce(psum_pool, acc, res, op, jbase, dt=F32):
                # acc [128, n*128] -> res [128, jbase:jbase+n] token-part
                n = acc.shape[-1] // 128
                for j in range(n):
                    trp = psum_pool.tile([128, 128], dt, tag="tr", bufs=2,
                                         name="trp")
                    nc.tensor.transpose(trp[:, :], acc[:, ts(j, 128)],
                                        ident_b[:, :] if dt == BF16
                                        else ident[:, :])
                    nc.vector.tensor_reduce(
                        out=res[:, jbase + j:jbase + j + 1], in_=trp[:, :],
                        axis=AX, op=op)

            def finalize(Mx, ssq, r0, sqrt_dim, mclip_col):
                """Mx/ssq [128, TJ] token-part absmax / sumsq -> stat rows
                r0 (qs) and r0+1 (al)."""
                nrm = stats.tile([128, TJ], F32, name="nrm")
                nc.vector.tensor_scalar(nrm, ssq, 1e-38, None, OP.max)
                nc.scalar.activation(nrm, nrm, AF.Sqrt)
                nc.vector.tensor_scalar(nrm, nrm, 1e-12, None, OP.max)
                inv_n = stats.tile([128, TJ], F32, name="inv_n")
                nc.vector.reciprocal(inv_n, nrm)
                al = stats.tile([128, TJ], F32, name="al")
                nc.vector.tensor_tensor(al, Mx, inv_n, OP.mult)
                nc.vector.tensor_scalar(al, al, sqrt_dim, 1e-5,
                                        OP.mult, OP.max)
                if mclip_col is None:
                    nc.vector.tensor_scalar(al, al, 1.0 / 127.0, None,
                                            OP.mult)
                else:
                    nc.vector.tensor_scalar(
                        al, al, wsc[:, mclip_col:mclip_col + 1], 1.0 / 127.0,
                        OP.mult, OP.mult)
                qs = stats.tile([128, TJ], F32, name="qs")
                nc.vector.tensor_scalar(qs, Mx, 1e-30, None, OP.max)
                nc.vector.reciprocal(qs, qs)
                nc.vector.tensor_scalar(qs, qs, 127.0, None, OP.mult)
                nc.sync.dma_start(out=stok_v[r0], in_=qs[:, :])
                nc.sync.dma_start(out=stok_v[r0 + 1], in_=al[:, :])

            def broadcast_row(row, dst):
                tmp = stats.tile([1, T], F32, name=f"brow{row}")
                nc.sync.dma_start(out=tmp[:, :], in_=srow_v[row:row + 1, :])
                nc.gpsimd.partition_broadcast(dst[:, :], tmp[:, :])

            def quant_wtile(src_f32, dst_q, scol, wf):
                """ternary-quantize f32 -> fp8 on ACT+DVE (JIT path).
                scol: wsc column holding s = 1/mclip."""
                nc.scalar.activation(wf, src_f32, AF.Copy,
                                     scale=wsc[:, scol:scol + 1])
                nc.vector.tensor_scalar(wf, wf, W_CLIP, -W_CLIP,
                                        OP.min, OP.max)
                nc.vector.tensor_scalar(dst_q, wf, C_ROUND, C_ROUND,
                                        OP.add, OP.subtract)

            def gq_quant(src_f32, dst_q, mcol, ta, tb):
                """ternary-quantize on GpSimd only: t = src/mclip; round and
                clip via (t+C) clamped to [C-1, C+1], minus C."""
                nc.gpsimd.tensor_scalar(ta, src_f32,
                                        wsc[:, mcol:mcol + 1], C_ROUND,
                                        OP.divide, OP.add)
                nc.gpsimd.tensor_scalar(tb, ta, C_ROUND - 1.0, C_ROUND + 1.0,
                                        OP.max, OP.min)
                nc.gpsimd.tensor_scalar(dst_q, tb, C_ROUND, None,
                                        OP.subtract)

            with (
                tc.tile_pool(name="xqp", bufs=1) as xqp,
                tc.tile_pool(name="wq", bufs=1) as wq,
            ):
                xq = xqp.tile([128, KD, T], BF16, name="xq")
                qs1_b = xqp.tile([128, T], F32, name="qs1_b")
                alx_b = xqp.tile([128, T], F32, name="alx_b")

                # ============ prologue ============
                with (
                    tc.tile_pool(name="pre", bufs=2) as pre,
                    tc.tile_pool(name="ps1", bufs=4, space="PSUM") as psum1,
                ):
                    # --- |w| shard partials (DVE reduce at queue head,
                    #     GpSimd partition sum) -> AllReduce ---
                    wcols = stats.tile([128, 16], F32, name="wcols")
                    for s in range(SO1):
                        wt1 = wq.tile([128, KD * 128], F32, tag="wldf",
                                      bufs=2, name="wt1")
                        nc.sync.dma_start(
                            out=wt1.rearrange("p (k j) -> p k j", j=128),
                            in_=w1s_t[s])
                        nc.vector.tensor_reduce(
                            out=wcols[:, s:s + 1], in_=wt1, axis=AX,
                            op=OP.add, apply_absolute_value=True)
                    nw2 = MO2 * W2C
                    for m in range(MO2):
                        for cpos in range(W2C):
                            wt2 = wq.tile([128, K2C * 128], F32, tag="wldf",
                                          bufs=2, name="wt2")
                            nc.sync.dma_start(
                                out=wt2.rearrange("p (k j) -> p k j", j=128),
                                in_=w2s_t[m][:, ts(cpos, K2C)])
                            nc.vector.tensor_reduce(
                                out=wcols[:, 8 + m * W2C + cpos:
                                          9 + m * W2C + cpos],
                                in_=wt2, axis=AX,
                                op=OP.add, apply_absolute_value=True)
                    wpad = stats.tile([128, 128], F32, name="wpad")
                    nc.vector.memset(wpad, 0.0)
                    nc.vector.reduce_sum(wpad[:, 0:1], wcols[:, 0:SO1],
                                         axis=AX)
                    nc.vector.reduce_sum(wpad[:, 1:2], wcols[:, 8:8 + nw2],
                                         axis=AX)
                    trw = psum1.tile([128, 128], F32, tag="tr", bufs=2,
                                     name="trw")
                    nc.tensor.transpose(trw[:, :], wpad[:, :], ident[:, :])
                    wred = stats.tile([8, 1], F32, name="wred")
                    nc.vector.memset(wred, 0.0)
                    nc.vector.reduce_sum(wred[0:2, :], trw[0:2, :], axis=AX)
                    nc.sync.dma_start(out=ar_in.ap()[0:8], in_=wred[:, :])
                    nc.gpsimd.collective_compute(
                        "AllReduce", OP.add,
                        replica_groups=[list(range(n_cores))],
                        ins=[ar_in.ap().opt()], outs=[ar_out.ap().opt()])

                    # --- x stats (ACT+DVE, overlap the AllReduce) ---
                    am1 = pre.tile([128, T], F32, tag="xst", bufs=2,
                                   name="am1")
                    sq1 = pre.tile([128, T], F32, tag="xst", bufs=2,
                                   name="sq1")
                    for k in range(KD):
                        xk = pre.tile([128, T], F32, tag="xk", bufs=3,
                                      name="xk")
                        nc.sync.dma_start(out=xk[:, :], in_=xT_t[k])
                        xab = pre.tile([128, T], F32, tag="xab", name="xab")
                        nc.scalar.activation(xab, xk, AF.Abs)
                        xsq = pre.tile([128, T], F32, tag="xsq", name="xsq")
                        nc.scalar.activation(xsq, xk, AF.Square)
                        if k == 0:
                            nc.vector.tensor_copy(am1, xab)
                            nc.vector.tensor_copy(sq1, xsq)
                        else:
                            nc.vector.tensor_tensor(am1, xab, am1, OP.max)
                            nc.vector.tensor_tensor(sq1, xsq, sq1, OP.add)
                    Mx1 = stats.tile([128, TJ], F32, name="Mx1")
                    part_reduce(psum1, am1, Mx1, OP.max, 0)
                    Sq1 = stats.tile([128, TJ], F32, name="Sq1")
                    part_reduce(psum1, sq1, Sq1, OP.add, 0)
                    finalize(Mx1, Sq1, 0, sqrt_d, None)
                    broadcast_row(0, qs1_b)
                    broadcast_row(1, alx_b)

                    # --- weight scale finalize (AllReduce done by now) ---
                    wrow = stats.tile([1, 4], F32, name="wrow")
                    nc.sync.dma_start(out=wrow[:, 0:2], in_=ar_out.ap()[0:2])
                    nc.vector.tensor_scalar(wrow[:, 2:4], wrow[:, 0:2],
                                            inv_cnt, 1e-5, OP.mult, OP.max)
                    nc.gpsimd.partition_broadcast(wsc[:, 0:2], wrow[:, 2:4])
                    nc.vector.reciprocal(wsc[:, 2:4], wsc[:, 0:2])

                    # --- xq: int8 values in bf16 ---
                    for k in range(KD):
                        xk2 = pre.tile([128, T], F32, tag="xk", bufs=3,
                                       name="xk2")
                        nc.sync.dma_start(out=xk2[:, :], in_=xT_t[k])
                        nc.vector.tensor_tensor(xk2, xk2, qs1_b, OP.mult)
                        nc.vector.tensor_scalar(xq[:, k, :], xk2, C_ROUND,
                                                C_ROUND, OP.add, OP.subtract)

                    # --- w1 shard quant -> AllGather (AllReduce done) ---
                    for s in range(SO1):
                        wq1f = wq.tile([128, KD * 128], F32, tag="wldf",
                                       bufs=2, name="wq1f")
                        nc.sync.dma_start(
                            out=wq1f.rearrange("p (k j) -> p k j", j=128),
                            in_=w1s_t[s])
                        wq8 = wq.tile([128, KD * 128], FP8, tag="wq8",
                                      bufs=2, name="wq8")
                        quant_wtile(wq1f, wq8, 2, wq1f)
                        nc.sync.dma_start(out=w1ag_in.ap()[s],
                                          in_=wq8[:, :])
                    nc.gpsimd.collective_compute(
                        "AllGather", OP.bypass,
                        replica_groups=[list(range(n_cores))],
                        ins=[w1ag_in.ap().opt()], outs=[w1q.ap().opt()])

                # ===================== L1 =====================
                # (w2 shard quant is interleaved into the strip loop below,
                # so its AllReduce dependency is long met by the time the
                # in-order engine queues reach it)
                with (
                    tc.tile_pool(name="l1", bufs=2) as l1,
                    tc.tile_pool(name="psl1", bufs=6, space="PSUM") as psl1,
                ):
                    amA = l1.tile([128, TH], F32, tag="st", bufs=2,
                                  name="amA")
                    amB = l1.tile([128, TH], F32, tag="st", bufs=2,
                                  name="amB")
                    sqA = l1.tile([128, TH], BF16, tag="stq", bufs=2,
                                  name="sqA")
                    sqB = l1.tile([128, TH], BF16, tag="stq", bufs=2,
                                  name="sqB")
                    for si in range(KI):
                        if si < j_jit:
                            w1f = l1.tile([128, KD * 128], F32, tag="w1f",
                                          bufs=3, name="w1f")
                            nc.sync.dma_start(
                                out=w1f.rearrange("p (k j) -> p k j", j=128),
                                in_=w1_t[si])
                            w1k = l1.tile([128, KD, 128], FP8, tag="w1k",
                                          bufs=3, name="w1k")
                            quant_wtile(w1f,
                                        w1k.rearrange("p k j -> p (k j)"),
                                        2, w1f)
                        else:
                            w1k = l1.tile([128, KD, 128], FP8, tag="w1k",
                                          bufs=3, name="w1k")
                            nc.sync.dma_start(
                                out=w1k.rearrange("p k j -> p (k j)"),
                                in_=w1q.ap()[si])
                        psA = psl1.tile([128, TH], F32, tag="mm", name="psA")
                        psB = psl1.tile([128, TH], F32, tag="mm", name="psB")
                        for k in range(KD):
                            nc.tensor.matmul(psA[:, :], w1k[:, k, :],
                                             xq[:, k, 0:TH],
                                             start=(k == 0),
                                             stop=(k == KD - 1))
                        for k in range(KD):
                            nc.tensor.matmul(psB[:, :], w1k[:, k, :],
                                             xq[:, k, TH:T],
                                             start=(k == 0),
                                             stop=(k == KD - 1))
                        for ch, ps, am, sq in ((0, psA, amA, sqA),
                                               (1, psB, amB, sqB)):
                            nc.vector.tensor_tensor(
                                ps, ps, alx_b[:, ts(ch, TH)], OP.mult)
                            if si == 0:
                                nc.vector.tensor_copy(am, ps)
                            else:
                                nc.vector.tensor_tensor(am, ps, am, OP.max)
                            hsb = l1.tile([128, TH], F32, tag="hsb", bufs=4,
                                          name="hsb")
                            gelu(hsb, ps, wsc[:, 0:1])
                            nc.sync.dma_start(
                                out=h_dram.ap()[si][:, ts(ch, TH)],
                                in_=hsb[:, :])
                            hsq = l1.tile([128, TH], BF16, tag="hsq",
                                          bufs=4, name="hsq")
                            nc.scalar.activation(hsq, hsb, AF.Square)
                            if si == 0:
                                nc.vector.tensor_copy(sq, hsq)
                            else:
                                nc.vector.tensor_tensor(sq, hsq, sq, OP.add)

                        # ---- interleaved w2 shard quant -> AllGather ----
                        nw2 = MO2 * W2C
                        W2BASE = min(j_jit, KI - 2 * nw2)
                        if si >= W2BASE and si < W2BASE + 2 * nw2 and \
                                (si - W2BASE) % 2 == 0:
                            ci = (si - W2BASE) // 2
                            m, cpos = divmod(ci, W2C)
                            wq2f = wq.tile([128, K2C * 128], F32,
                                           tag="wldf", bufs=2, name="wq2f")
                            nc.sync.dma_start(
                                out=wq2f.rearrange("p (k j) -> p k j",
                                                   j=128),
                                in_=w2s_t[m][:, ts(cpos, K2C)])
                            wq8b = wq.tile([128, K2C * 128], FP8, tag="wq8",
                                           bufs=2, name="wq8b")
                            quant_wtile(wq2f, wq8b, 3, wq2f)
                            nc.sync.dma_start(
                                out=w2ag_in.ap()[m][:, ts(cpos, K2C * 128)],
                                in_=wq8b[:, :])
                            if ci == nw2 - 1:
                                nc.gpsimd.collective_compute(
                                    "AllGather", OP.bypass,
                                    replica_groups=[list(range(n_cores))],
                                    ins=[w2ag_in.ap().opt()],
                                    outs=[w2q.ap().opt()])

                    # -------- h stats finalize --------
                    MxH = stats.tile([128, TJ], F32, name="MxH")
                    part_reduce(psl1, amA, MxH, OP.max, 0)
                    part_reduce(psl1, amB, MxH, OP.max, TJH)
                    SqH = stats.tile([128, TJ], F32, name="SqH")
                    part_reduce(psl1, sqA, SqH, OP.add, 0, dt=BF16)
                    part_reduce(psl1, sqB, SqH, OP.add, TJH, dt=BF16)
                    # absmax(h) = gelu(mclip1 * max(alpha*psum)): monotone
                    gelu(MxH, MxH, wsc[:, 0:1])
                    finalize(MxH, SqH, 2, sqrt_i, 1)
                    broadcast_row(2, qs2_b)
                    broadcast_row(3, al2_b)

            # ===================== hq + L2 =====================
            with (
                tc.tile_pool(name="l2", bufs=2) as l2,
                tc.tile_pool(name="psl2", bufs=8, space="PSUM") as psl2,
            ):
                hq = l2.tile([128, KI, T], BF16, bufs=1, name="hq")
                for k2 in range(KI):
                    hk = l2.tile([128, T], F32, tag="hk", bufs=3, name="hk")
                    nc.sync.dma_start(out=hk[:, :], in_=h_dram.ap()[k2])
                    nc.vector.tensor_tensor(hk, hk, qs2_b, OP.mult)
                    nc.vector.tensor_scalar(hq[:, k2, :], hk, C_ROUND,
                                            C_ROUND, OP.add, OP.subtract)

                for g in range(MD // MG):
                    wg = []
                    for mi in range(MG):
                        m = g * MG + mi
                        w2k = l2.tile([128, KI, 128], FP8, tag="w2k",
                                      bufs=4, name="w2k")
                        nc.sync.dma_start(
                            out=w2k.rearrange("p k j -> p (k j)"),
                            in_=w2q.ap()[m])
                        wg.append(w2k)
                    pb = [psl2.tile([128, TH], F32, tag="mm",
                                    name=f"pb{g}_{q}") for q in range(8)]
                    if g == 0:
                        # k2-outer: consume hq strips as they are built
                        for k2 in range(KI):
                            for mi in range(MG):
                                nc.tensor.matmul(pb[2 * mi][:, :],
                                                 wg[mi][:, k2, :],
                                                 hq[:, k2, 0:TH],
                                                 start=(k2 == 0),
                                                 stop=(k2 == KI - 1))
                                nc.tensor.matmul(pb[2 * mi + 1][:, :],
                                                 wg[mi][:, k2, :],
                                                 hq[:, k2, TH:T],
                                                 start=(k2 == 0),
                                                 stop=(k2 == KI - 1))
                    else:
                        # same-bank chains: ~50ns/MM cheaper than rotating
                        for mi in range(MG):
                            for ch in range(2):
                                for k2 in range(KI):
                                    nc.tensor.matmul(
                                        pb[2 * mi + ch][:, :],
                                        wg[mi][:, k2, :],
                                        hq[:, k2, ts(ch, TH)],
                                        start=(k2 == 0),
                                        stop=(k2 == KI - 1))
                    for mi in range(MG):
                        m = g * MG + mi
                        for ch in range(2):
                            ob = l2.tile([128, TH], F32, tag="ob", bufs=2,
                                         name="ob")
                            nc.vector.tensor_tensor(
                                ob, pb[2 * mi + ch], al2_b[:, ts(ch, TH)],
                                OP.mult)
                            nc.sync.dma_start(
                                out=out_w[m][:, ts(ch, TH)], in_=ob[:, :])

    nc.compile()
    return nc


_NC_CACHE = {}


def _get_nc(D, I, T, n_cores, **kw):
    key = (D, I, T, n_cores, tuple(sorted(kw.items())))
    if key not in _NC_CACHE:
        _NC_CACHE[key] = build_bitnet(D, I, T, n_cores, **kw)
    return _NC_CACHE[key]


def make_in_maps(x, w1, w2, n_cores=N_CORES):
    """Host-side sharding/layout only (transpose + slicing, no arithmetic)."""
    xf = np.ascontiguousarray(np.asarray(x, dtype=np.float32)).reshape(
        -1, x.shape[-1])
    D = xf.shape[1]
    I = w1.shape[0]
    T = xf.shape[0] // n_cores
    w1T = np.ascontiguousarray(np.asarray(w1, dtype=np.float32).T)  # [D, I]
    w2T = np.ascontiguousarray(np.asarray(w2, dtype=np.float32).T)  # [I, D]
    O1 = I // n_cores
    O2 = D // n_cores
    in_maps = []
    for c in range(n_cores):
        xTc = np.ascontiguousarray(xf[c * T:(c + 1) * T].T)  # [D, T]
        in_maps.append({
            "xT": xTc,
            "w1T": w1T,
            "w1s": np.ascontiguousarray(w1T[:, c * O1:(c + 1) * O1]),
            "w2s": np.ascontiguousarray(w2T[:, c * O2:(c + 1) * O2]),
        })
    return in_maps, (D, I, T)


def run_spmd(x, w1, w2, trace=False, **kwargs):
    from concourse.bass_utils import run_bass_kernel_spmd

    B, S, D = x.shape
    in_maps, (D, I, T) = make_in_maps(x, w1, w2, N_CORES)
    nc = _get_nc(D, I, T, N_CORES)
    res = run_bass_kernel_spmd(nc, in_maps, core_ids=list(range(N_CORES)),
                               trace=trace, **kwargs)
    outs = [res.results[c]["outT"].T for c in range(N_CORES)]  # each [T, D]
    out = np.concatenate(outs, axis=0).reshape(B, S, D)
    return np.ascontiguousarray(out, dtype=np.float32), res


def kernel(x, w1, w2):
    out, _ = run_spmd(x, w1, w2, trace=False)
    return out


# revision 5
# speedup vs baseline: 1.0819x; 1.0460x over previous
"""BitNet FFN Trainium2 kernel, v4.

Data-parallel over tokens (1024/core). Ternary weights in fp8e4 (exact),
int8 activations in bf16 (exact); matmuls mixed fp8 lhsT x bf16 rhs
(HW-validated exact). Per-token dequant scale applied to PSUM on DVE;
mean|w| rides the ACT Gelu scale input.

Engine assignment (strict-FIFO queues make this the core design axis):
  DVE:    x stats trees, xq build, JIT clamp/round, PSUM alpha/max/sq,
          hq mult, L2 out scale
  ACT:    x abs/sq, JIT scale-copy, gelu, squares
  GpSimd: |w| shard reduces, AllReduce pack, weight-scale bcasts, shard
          ternary quant (divide recipe), collectives, hq round
  PE:     matmuls (same-bank chains), stat transposes
"""

import sys

for _p in ("/opt/trn_rl_repo", "/opt/trn_rl_repo/concourse"):
    if _p not in sys.path:
        sys.path.insert(0, _p)

import numpy as np

import concourse.bass as bass
import concourse.bacc as bacc
import concourse.mybir as mybir
import concourse.tile as tile
from concourse import bass_isa
from concourse.bass import ts
from concourse.masks import make_identity

F32 = mybir.dt.float32
BF16 = mybir.dt.bfloat16
FP8 = mybir.dt.float8e4
AX = mybir.AxisListType.X
OP = mybir.AluOpType
AF = mybir.ActivationFunctionType

C_ROUND = 12582912.0  # 1.5 * 2**23 : fp32 RNE rounding constant
W_CLIP = 1.4999999    # round(clamp(t, +-W_CLIP)) == clip(round(t), -1, 1)
N_CORES = 8
J_JIT = 16            # L1 strips quantized locally while AllGather in flight
N_EARLY = 3           # strips whose quant is emitted ahead of the xq loop


def build_bitnet(D, I, T, n_cores=N_CORES, gelu_mode="gelu", j_jit=J_JIT):
    KD = D // 128        # contraction tiles for L1
    KI = I // 128        # strips == L2 contraction tiles
    MD = D // 128        # output m-tiles for L2
    TH = T // 2          # token chunk
    TJ = T // 128        # stat columns (full T)
    TJH = TH // 128      # stat columns per chunk
    SO1 = I // n_cores // 128   # own w1 strips
    MO2 = D // 128 // n_cores   # own w2 m-tiles
    MG = 4                      # L2 m-tiles per pass (MG*2 psum banks)
    W2C = max(1, KI // 16)      # w2 shard chunks per m ([128, <=2048] each)
    K2C = KI // W2C
    inv_cnt = 1.0 / float(D * I)
    sqrt_d = float(np.sqrt(np.float64(D)))
    sqrt_i = float(np.sqrt(np.float64(I)))
    j_jit = min(j_jit, KI)

    nc = bacc.Bacc("TRN2", num_devices=n_cores)

    xT = nc.dram_tensor("xT", [D, T], F32, kind="ExternalInput")
    w1T = nc.dram_tensor("w1T", [D, I], F32, kind="ExternalInput")
    w1s = nc.dram_tensor("w1s", [D, I // n_cores], F32, kind="ExternalInput")
    w2s = nc.dram_tensor("w2s", [I, D // n_cores], F32, kind="ExternalInput")
    outT = nc.dram_tensor("outT", [D, T], F32, kind="ExternalOutput")

    h_dram = nc.dram_tensor("h_scratch", [KI, 128, T], F32, kind="Internal")
    w1ag_in = nc.dram_tensor("w1ag_in", [SO1, 128, KD * 128], FP8,
                             kind="Internal")
    w1q = nc.dram_tensor("w1q_ag", [KI, 128, KD * 128], FP8, kind="Internal",
                         addr_space="Shared")
    w2ag_in = nc.dram_tensor("w2ag_in", [MO2, 128, KI * 128], FP8,
                             kind="Internal")
    w2q = nc.dram_tensor("w2q_ag", [MD, 128, KI * 128], FP8, kind="Internal",
                         addr_space="Shared")
    ar_in = nc.dram_tensor("ar_in", [8], F32, kind="Internal")
    ar_out = nc.dram_tensor("ar_out", [8], F32, kind="Internal",
                            addr_space="Shared")
    stat_dram = nc.dram_tensor("stat_dram", [4, T], F32, kind="Internal")
    srow_v = stat_dram.ap()                                       # [4, T]
    stok_v = stat_dram.ap().rearrange("r (j p) -> r p j", p=128)  # [4,128,TJ]

    xT_t = xT.ap().rearrange("(k p) t -> k p t", p=128)           # [KD,128,T]
    w1_t = w1T.ap().rearrange("(k p) (s j) -> s p k j", p=128, j=128)
    w1s_t = w1s.ap().rearrange("(k p) (s j) -> s p k j", p=128, j=128)
    w2s_t = w2s.ap().rearrange("(k p) (m j) -> m p k j", p=128, j=128)
    out_w = outT.ap().rearrange("(m p) t -> m p t", p=128)

    def gelu(out, in_, scale):
        if gelu_mode == "gelu":
            nc.scalar.activation(out, in_, AF.Gelu, scale=scale)
        else:  # CoreSim lacks the Gelu table; sim-only stand-in
            nc.scalar.activation(out, in_, AF.Sigmoid, scale=scale)

    with tile.TileContext(nc) as tc:
        with (
            tc.tile_pool(name="glob", bufs=1) as glob,
            tc.tile_pool(name="stats", bufs=1) as stats,
        ):
            ident = glob.tile([128, 128], F32)
            make_identity(nc, ident)
            ident_b = glob.tile([128, 128], BF16)
            make_identity(nc, ident_b)
            wsc = glob.tile([128, 4], F32)   # cols: mclip1, mclip2, s1, s2
            qs2_b = glob.tile([128, T], F32, name="qs2_b")
            al2_b = glob.tile([128, T], F32, name="al2_b")

            def part_reduce(psum_pool, acc, res, op, jbase, dt=F32):
                # acc [128, n*128] -> res [128, jbase:jbase+n] token-part
                n = acc.shape[-1] // 128
                for j in range(n):
                    trp = psum_pool.tile([128, 128], dt, tag="tr", bufs=2,
                                         name="trp")
                    nc.tensor.transpose(trp[:, :], acc[:, ts(j, 128)],
                                        ident_b[:, :] if dt == BF16
                                        else ident[:, :])
                    nc.vector.tensor_reduce(
                        out=res[:, jbase + j:jbase + j + 1], in_=trp[:, :],
                        axis=AX, op=op)

            def finalize(Mx, ssq, r0, sqrt_dim, mclip_col):
                """Mx/ssq [128, TJ] token-part absmax / sumsq -> stat rows
                r0 (qs) and r0+1 (al)."""
                nrm = stats.tile([128, TJ], F32, name="nrm")
                nc.vector.tensor_scalar(nrm, ssq, 1e-38, None, OP.max)
                nc.scalar.activation(nrm, nrm, AF.Sqrt)
                nc.vector.tensor_scalar(nrm, nrm, 1e-12, None, OP.max)
                inv_n = stats.tile([128, TJ], F32, name="inv_n")
                nc.vector.reciprocal(inv_n, nrm)
                al = stats.tile([128, TJ], F32, name="al")
                nc.vector.tensor_tensor(al, Mx, inv_n, OP.mult)
                nc.vector.tensor_scalar(al, al, sqrt_dim, 1e-5,
                                        OP.mult, OP.max)
                if mclip_col is None:
                    nc.vector.tensor_scalar(al, al, 1.0 / 127.0, None,
                                            OP.mult)
                else:
                    nc.vector.tensor_scalar(
                        al, al, wsc[:, mclip_col:mclip_col + 1], 1.0 / 127.0,
                        OP.mult, OP.mult)
                qs = stats.tile([128, TJ], F32, name="qs")
                nc.vector.tensor_scalar(qs, Mx, 1e-30, None, OP.max)
                nc.vector.reciprocal(qs, qs)
                nc.vector.tensor_scalar(qs, qs, 127.0, None, OP.mult)
                nc.sync.dma_start(out=stok_v[r0], in_=qs[:, :])
                nc.sync.dma_start(out=stok_v[r0 + 1], in_=al[:, :])

            def broadcast_row(row, dst):
                tmp = stats.tile([1, T], F32, name=f"brow{row}")
                nc.sync.dma_start(out=tmp[:, :], in_=srow_v[row:row + 1, :])
                nc.gpsimd.partition_broadcast(dst[:, :], tmp[:, :])

            def quant_wtile(src_f32, dst_q, scol, wf):
                """ternary-quantize f32 -> fp8 on ACT+DVE (JIT path).
                scol: wsc column holding s = 1/mclip."""
                nc.scalar.activation(wf, src_f32, AF.Copy,
                                     scale=wsc[:, scol:scol + 1])
                nc.vector.tensor_scalar(wf, wf, W_CLIP, -W_CLIP,
                                        OP.min, OP.max)
                nc.vector.tensor_scalar(dst_q, wf, C_ROUND, C_ROUND,
                                        OP.add, OP.subtract)

            def gq_quant(src_f32, dst_q, mcol, ta, tb):
                """ternary-quantize on GpSimd only: t = src/mclip; round and
                clip via (t+C) clamped to [C-1, C+1], minus C."""
                nc.gpsimd.tensor_scalar(ta, src_f32,
                                        wsc[:, mcol:mcol + 1], C_ROUND,
                                        OP.divide, OP.add)
                nc.gpsimd.tensor_scalar(tb, ta, C_ROUND - 1.0, C_ROUND + 1.0,
                                        OP.max, OP.min)
                nc.gpsimd.tensor_scalar(dst_q, tb, C_ROUND, None,
                                        OP.subtract)

            with (
                tc.tile_pool(name="xqp", bufs=1) as xqp,
                tc.tile_pool(name="wq", bufs=1) as wq,
            ):
                xq = xqp.tile([128, KD, T], BF16, name="xq")
                qs1_b = xqp.tile([128, T], F32, name="qs1_b")
                alx_b = xqp.tile([128, T], F32, name="alx_b")

                # ============ prologue ============
                with (
                    tc.tile_pool(name="pre", bufs=2) as pre,
                    tc.tile_pool(name="ps1", bufs=4, space="PSUM") as psum1,
                ):
                    # --- |w| shard partials (DVE reduce at queue head,
                    #     GpSimd partition sum) -> AllReduce ---
                    wcols = stats.tile([128, 16], F32, name="wcols")
                    for s in range(SO1):
                        wt1 = wq.tile([128, KD * 128], F32, tag="wldf",
                                      bufs=2, name="wt1")
                        nc.sync.dma_start(
                            out=wt1.rearrange("p (k j) -> p k j", j=128),
                            in_=w1s_t[s])
                        nc.vector.tensor_reduce(
                            out=wcols[:, s:s + 1], in_=wt1, axis=AX,
                            op=OP.add, apply_absolute_value=True)
                    nw2 = MO2 * W2C
                    for m in range(MO2):
                        for cpos in range(W2C):
                            wt2 = wq.tile([128, K2C * 128], F32, tag="wldf",
                                          bufs=2, name="wt2")
                            nc.sync.dma_start(
                                out=wt2.rearrange("p (k j) -> p k j", j=128),
                                in_=w2s_t[m][:, ts(cpos, K2C)])
                            nc.vector.tensor_reduce(
                                out=wcols[:, 8 + m * W2C + cpos:
                                          9 + m * W2C + cpos],
                                in_=wt2, axis=AX,
                                op=OP.add, apply_absolute_value=True)
                    wpad = stats.tile([128, 128], F32, name="wpad")
                    nc.vector.memset(wpad, 0.0)
                    nc.vector.reduce_sum(wpad[:, 0:1], wcols[:, 0:SO1],
                                         axis=AX)
                    nc.vector.reduce_sum(wpad[:, 1:2], wcols[:, 8:8 + nw2],
                                         axis=AX)
                    trw = psum1.tile([128, 128], F32, tag="tr", bufs=2,
                                     name="trw")
                    nc.tensor.transpose(trw[:, :], wpad[:, :], ident[:, :])
                    wred = stats.tile([8, 1], F32, name="wred")
                    nc.vector.memset(wred, 0.0)
                    nc.vector.reduce_sum(wred[0:2, :], trw[0:2, :], axis=AX)
                    nc.sync.dma_start(out=ar_in.ap()[0:8], in_=wred[:, :])
                    nc.gpsimd.collective_compute(
                        "AllReduce", OP.add,
                        replica_groups=[list(range(n_cores))],
                        ins=[ar_in.ap().opt()], outs=[ar_out.ap().opt()])

                    # --- x stats (ACT+DVE, overlap the AllReduce) ---
                    am1 = pre.tile([128, T], F32, tag="xst", bufs=2,
                                   name="am1")
                    sq1 = pre.tile([128, T], F32, tag="xst", bufs=2,
                                   name="sq1")
                    for k in range(KD):
                        xk = pre.tile([128, T], F32, tag="xk", bufs=3,
                                      name="xk")
                        nc.sync.dma_start(out=xk[:, :], in_=xT_t[k])
                        xab = pre.tile([128, T], F32, tag="xab", name="xab")
                        nc.scalar.activation(xab, xk, AF.Abs)
                        xsq = pre.tile([128, T], F32, tag="xsq", name="xsq")
                        nc.scalar.activation(xsq, xk, AF.Square)
                        if k == 0:
                            nc.vector.tensor_copy(am1, xab)
                            nc.vector.tensor_copy(sq1, xsq)
                        else:
                            nc.vector.tensor_tensor(am1, xab, am1, OP.max)
                            nc.vector.tensor_tensor(sq1, xsq, sq1, OP.add)
                    Mx1 = stats.tile([128, TJ], F32, name="Mx1")
                    part_reduce(psum1, am1, Mx1, OP.max, 0)
                    Sq1 = stats.tile([128, TJ], F32, name="Sq1")
                    part_reduce(psum1, sq1, Sq1, OP.add, 0)
                    finalize(Mx1, Sq1, 0, sqrt_d, None)
                    broadcast_row(0, qs1_b)
                    broadcast_row(1, alx_b)

                    # --- weight scale finalize (AllReduce done by now) ---
                    wrow = stats.tile([1, 4], F32, name="wrow")
                    nc.sync.dma_start(out=wrow[:, 0:2], in_=ar_out.ap()[0:2])
                    nc.vector.tensor_scalar(wrow[:, 2:4], wrow[:, 0:2],
                                            inv_cnt, 1e-5, OP.mult, OP.max)
                    nc.gpsimd.partition_broadcast(wsc[:, 0:2], wrow[:, 2:4])
                    nc.vector.reciprocal(wsc[:, 2:4], wsc[:, 0:2])

                    # --- early L1 strips' quant, ahead of xq in the DVE
                    #     queue so their matmuls start as xq slices land ---
                    n_early = min(N_EARLY, j_jit)
                    early_w1k = []
                    for s in range(n_early):
                        ew1f = wq.tile([128, KD * 128], F32, tag="wldf",
                                       bufs=2, name="ew1f")
                        nc.sync.dma_start(
                            out=ew1f.rearrange("p (k j) -> p k j", j=128),
                            in_=w1_t[s])
                        ew1k = wq.tile([128, KD, 128], FP8, tag="ew1k",
                                       bufs=N_EARLY, name="ew1k")
                        quant_wtile(ew1f,
                                    ew1k.rearrange("p k j -> p (k j)"),
                                    2, ew1f)
                        early_w1k.append(ew1k)

                    # --- xq: int8 values in bf16 ---
                    for k in range(KD):
                        xk2 = pre.tile([128, T], F32, tag="xk", bufs=3,
                                       name="xk2")
                        nc.sync.dma_start(out=xk2[:, :], in_=xT_t[k])
                        nc.vector.tensor_tensor(xk2, xk2, qs1_b, OP.mult)
                        nc.vector.tensor_scalar(xq[:, k, :], xk2, C_ROUND,
                                                C_ROUND, OP.add, OP.subtract)

                    # --- w1 shard quant -> AllGather (AllReduce done) ---
                    for s in range(SO1):
                        wq1f = wq.tile([128, KD * 128], F32, tag="wldf",
                                       bufs=2, name="wq1f")
                        nc.sync.dma_start(
                            out=wq1f.rearrange("p (k j) -> p k j", j=128),
                            in_=w1s_t[s])
                        wq8 = wq.tile([128, KD * 128], FP8, tag="wq8",
                                      bufs=2, name="wq8")
                        quant_wtile(wq1f, wq8, 2, wq1f)
                        nc.sync.dma_start(out=w1ag_in.ap()[s],
                                          in_=wq8[:, :])
                    nc.gpsimd.collective_compute(
                        "AllGather", OP.bypass,
                        replica_groups=[list(range(n_cores))],
                        ins=[w1ag_in.ap().opt()], outs=[w1q.ap().opt()])

                # ===================== L1 =====================
                # (w2 shard quant is interleaved into the strip loop below,
                # so its AllReduce dependency is long met by the time the
                # in-order engine queues reach it)
                with (
                    tc.tile_pool(name="l1", bufs=2) as l1,
                    tc.tile_pool(name="psl1", bufs=6, space="PSUM") as psl1,
                ):
                    amA = l1.tile([128, TH], F32, tag="st", bufs=2,
                                  name="amA")
                    amB = l1.tile([128, TH], F32, tag="st", bufs=2,
                                  name="amB")
                    sqA = l1.tile([128, TH], BF16, tag="stq", bufs=2,
                                  name="sqA")
                    sqB = l1.tile([128, TH], BF16, tag="stq", bufs=2,
                                  name="sqB")
                    for si in range(KI):
                        if si < n_early:
                            w1k = early_w1k[si]
                        elif si < j_jit:
                            w1f = l1.tile([128, KD * 128], F32, tag="w1f",
                                          bufs=3, name="w1f")
                            nc.sync.dma_start(
                                out=w1f.rearrange("p (k j) -> p k j", j=128),
                                in_=w1_t[si])
                            w1k = l1.tile([128, KD, 128], FP8, tag="w1k",
                                          bufs=3, name="w1k")
                            quant_wtile(w1f,
                                        w1k.rearrange("p k j -> p (k j)"),
                                        2, w1f)
                        else:
                            w1k = l1.tile([128, KD, 128], FP8, tag="w1k",
                                          bufs=3, name="w1k")
                            nc.sync.dma_start(
                                out=w1k.rearrange("p k j -> p (k j)"),
                                in_=w1q.ap()[si])
                        psA = psl1.tile([128, TH], F32, tag="mm", name="psA")
                        psB = psl1.tile([128, TH], F32, tag="mm", name="psB")
                        for k in range(KD):
                            nc.tensor.matmul(psA[:, :], w1k[:, k, :],
                                             xq[:, k, 0:TH],
                                             start=(k == 0),
                                             stop=(k == KD - 1))
                        for k in range(KD):
                            nc.tensor.matmul(psB[:, :], w1k[:, k, :],
                                             xq[:, k, TH:T],
                                             start=(k == 0),
                                             stop=(k == KD - 1))
                        for ch, ps, am, sq in ((0, psA, amA, sqA),
                                               (1, psB, amB, sqB)):
                            nc.vector.tensor_tensor(
                                ps, ps, alx_b[:, ts(ch, TH)], OP.mult)
                            if si == 0:
                                nc.vector.tensor_copy(am, ps)
                            else:
                                nc.vector.tensor_tensor(am, ps, am, OP.max)
                            hsb = l1.tile([128, TH], F32, tag="hsb", bufs=4,
                                          name="hsb")
                            gelu(hsb, ps, wsc[:, 0:1])
                            nc.sync.dma_start(
                                out=h_dram.ap()[si][:, ts(ch, TH)],
                                in_=hsb[:, :])
                            hsq = l1.tile([128, TH], BF16, tag="hsq",
                                          bufs=4, name="hsq")
                            nc.scalar.activation(hsq, hsb, AF.Square)
                            if si == 0:
                                nc.vector.tensor_copy(sq, hsq)
                            else:
                                nc.vector.tensor_tensor(sq, hsq, sq, OP.add)

                        # ---- interleaved w2 shard quant -> AllGather ----
                        nw2 = MO2 * W2C
                        W2BASE = min(j_jit, KI - 2 * nw2)
                        if si >= W2BASE and si < W2BASE + 2 * nw2 and \
                                (si - W2BASE) % 2 == 0:
                            ci = (si - W2BASE) // 2
                            m, cpos = divmod(ci, W2C)
                            wq2f = wq.tile([128, K2C * 128], F32,
                                           tag="wldf", bufs=2, name="wq2f")
                            nc.sync.dma_start(
                                out=wq2f.rearrange("p (k j) -> p k j",
                                                   j=128),
                                in_=w2s_t[m][:, ts(cpos, K2C)])
                            wq8b = wq.tile([128, K2C * 128], FP8, tag="wq8",
                                           bufs=2, name="wq8b")
                            quant_wtile(wq2f, wq8b, 3, wq2f)
                            nc.sync.dma_start(
                                out=w2ag_in.ap()[m][:, ts(cpos, K2C * 128)],
                                in_=wq8b[:, :])
                            if ci == nw2 - 1:
                                nc.gpsimd.collective_compute(
                                    "AllGather", OP.bypass,
                                    replica_groups=[list(range(n_cores))],
                                    ins=[w2ag_in.ap().opt()],
                                    outs=[w2q.ap().opt()])

                    # -------- h stats finalize --------
                    MxH = stats.tile([128, TJ], F32, name="MxH")
                    part_reduce(psl1, amA, MxH, OP.max, 0)
                    part_reduce(psl1, amB, MxH, OP.max, TJH)
                    SqH = stats.tile([128, TJ], F32, name="SqH")
                    part_reduce(psl1, sqA, SqH, OP.add, 0, dt=BF16)
                    part_reduce(psl1, sqB, SqH, OP.add, TJH, dt=BF16)
                    # absmax(h) = gelu(mclip1 * max(alpha*psum)): monotone
                    gelu(MxH, MxH, wsc[:, 0:1])
                    finalize(MxH, SqH, 2, sqrt_i, 1)
                    broadcast_row(2, qs2_b)
                    broadcast_row(3, al2_b)

            # ===================== hq + L2 =====================
            with (
                tc.tile_pool(name="l2", bufs=2) as l2,
                tc.tile_pool(name="psl2", bufs=8, space="PSUM") as psl2,
            ):
                hq = l2.tile([128, KI, T], BF16, bufs=1, name="hq")
                for k2 in range(KI):
                    hk = l2.tile([128, T], F32, tag="hk", bufs=3, name="hk")
                    nc.sync.dma_start(out=hk[:, :], in_=h_dram.ap()[k2])
                    nc.vector.tensor_tensor(hk, hk, qs2_b, OP.mult)
                    nc.vector.tensor_scalar(hq[:, k2, :], hk, C_ROUND,
                                            C_ROUND, OP.add, OP.subtract)

                for g in range(MD // MG):
                    wg = []
                    for mi in range(MG):
                        m = g * MG + mi
                        w2k = l2.tile([128, KI, 128], FP8, tag="w2k",
                                      bufs=4, name="w2k")
                        nc.sync.dma_start(
                            out=w2k.rearrange("p k j -> p (k j)"),
                            in_=w2q.ap()[m])
                        wg.append(w2k)
                    pb = [psl2.tile([128, TH], F32, tag="mm",
                                    name=f"pb{g}_{q}") for q in range(8)]
                    if g == 0:
                        # k2-outer: consume hq strips as they are built
                        for k2 in range(KI):
                            for mi in range(MG):
                                nc.tensor.matmul(pb[2 * mi][:, :],
                                                 wg[mi][:, k2, :],
                                                 hq[:, k2, 0:TH],
                                                 start=(k2 == 0),
                                                 stop=(k2 == KI - 1))
                                nc.tensor.matmul(pb[2 * mi + 1][:, :],
                                                 wg[mi][:, k2, :],
                                                 hq[:, k2, TH:T],
                                                 start=(k2 == 0),
                                                 stop=(k2 == KI - 1))
                    else:
                        # same-bank chains: ~50ns/MM cheaper than rotating
                        for mi in range(MG):
                            for ch in range(2):
                                for k2 in range(KI):
                                    nc.tensor.matmul(
                                        pb[2 * mi + ch][:, :],
                                        wg[mi][:, k2, :],
                                        hq[:, k2, ts(ch, TH)],
                                        start=(k2 == 0),
                                        stop=(k2 == KI - 1))
                    for mi in range(MG):
                        m = g * MG + mi
                        for ch in range(2):
                            ob = l2.tile([128, TH], F32, tag="ob", bufs=2,
                                         name="ob")
                            nc.vector.tensor_tensor(
                                ob, pb[2 * mi + ch], al2_b[:, ts(ch, TH)],
                                OP.mult)
                            nc.sync.dma_start(
                                out=out_w[m][:, ts(ch, TH)], in_=ob[:, :])

    nc.compile()
    return nc


_NC_CACHE = {}


def _get_nc(D, I, T, n_cores, **kw):
    key = (D, I, T, n_cores, tuple(sorted(kw.items())))
    if key not in _NC_CACHE:
        _NC_CACHE[key] = build_bitnet(D, I, T, n_cores, **kw)
    return _NC_CACHE[key]


def make_in_maps(x, w1, w2, n_cores=N_CORES):
    """Host-side sharding/layout only (transpose + slicing, no arithmetic)."""
    xf = np.ascontiguousarray(np.asarray(x, dtype=np.float32)).reshape(
        -1, x.shape[-1])
    D = xf.shape[1]
    I = w1.shape[0]
    T = xf.shape[0] // n_cores
    w1T = np.ascontiguousarray(np.asarray(w1, dtype=np.float32).T)  # [D, I]
    w2T = np.ascontiguousarray(np.asarray(w2, dtype=np.float32).T)  # [I, D]
    O1 = I // n_cores
    O2 = D // n_cores
    in_maps = []
    for c in range(n_cores):
        xTc = np.ascontiguousarray(xf[c * T:(c + 1) * T].T)  # [D, T]
        in_maps.append({
            "xT": xTc,
            "w1T": w1T,
            "w1s": np.ascontiguousarray(w1T[:, c * O1:(c + 1) * O1]),
            "w2s": np.ascontiguousarray(w2T[:, c * O2:(c + 1) * O2]),
        })
    return in_maps, (D, I, T)


def run_spmd(x, w1, w2, trace=False, **kwargs):
    from concourse.bass_utils import run_bass_kernel_spmd

    B, S, D = x.shape
    in_maps, (D, I, T) = make_in_maps(x, w1, w2, N_CORES)
    nc = _get_nc(D, I, T, N_CORES)
    res = run_bass_kernel_spmd(nc, in_maps, core_ids=list(range(N_CORES)),
                               trace=trace, **kwargs)
    outs = [res.results[c]["outT"].T for c in range(N_CORES)]  # each [T, D]
    out = np.concatenate(outs, axis=0).reshape(B, S, D)
    return np.ascontiguousarray(out, dtype=np.float32), res


def kernel(x, w1, w2):
    out, _ = run_spmd(x, w1, w2, trace=False)
    return out
